# revision 18
# baseline (speedup 1.0000x reference)
"""Trainium2 Bass kernel for nn_Encoder_Baseline_28570122453316.

GRU-with-2-key-attention encoder. Data-parallel over batch: B=256 is split
32-per-core across 8 NeuronCores; the T=64 sequential scan runs locally on
each core. All attention key/value projections are hoisted out of the scan
(softmax over S=2 keys reduces to a sigmoid of a score difference), so the
scan itself only contains the hx @ [W_hh | Wq] matmul plus elementwise
gates.

Self-contained: hardcodes shapes, builds/compiles the Bass program on first
call, and runs it via run_bass_kernel_spmd on cores 0-7.
"""

import math
import sys

sys.path.insert(0, "/opt/trn_rl_repo")

import numpy as np
import ml_dtypes

import concourse.bass as bass
import concourse.tile as tile
from concourse import mybir
from concourse.tile import ScopedClock

F32 = mybir.dt.float32
BF16 = mybir.dt.bfloat16
AF = mybir.ActivationFunctionType
BF = ml_dtypes.bfloat16

B, T, DIN, DF, H = 256, 64, 2048, 2048, 512
NCORES = 8
BL = B // NCORES  # 32 batch rows per core
P = 128


class PatchedTileContext(tile.TileContext):
    """This walrus build caps sync-wait commands per instruction; the Tile
    exit drain carries one wait per ticked proc and overflows the cap.
    Split the waits onto single-wait SP NOPs emitted before the drain."""

    def _drain_and_barrier(self, tick_clock, wait_clock):
        probe = mybir.InstNoOp(name="tile_exit_wait_probe", engine=mybir.EngineType.SP)
        wait_clock.add_sem_waits(probe, ScopedClock({None: tick_clock.global_clock}))
        waits = list(probe.sync_info.on_wait) if probe.sync_info is not None else []
        for w in waits:
            ni = self.nc.sync.nop(nofuse=True)
            ni.ins.sync_info = mybir.SyncInfo(on_wait=[w], on_update=[])
        self.nc.sync.drain()
        self.nc.all_engine_barrier()
        assert self.sems is not None
        popped = self.nc._tile_sem_poison_stack.pop()
        assert popped is self._sem_poison
        self.nc.clear_and_free_semaphores(list(self.sems.allocated().values()))
        self.nc.all_engine_barrier()


_MAX_WAITS = 1


def _split_waits_bir(bir):
    """This walrus build rejects instructions carrying more than one sync
    wait. Hoist excess waits onto same-engine NoOps placed just before the
    instruction (same blocking semantics; engines execute in order)."""
    ctr = 0
    for f in bir["functions"]:
        for bb in f["blocks"]:
            new = []
            for ins in bb["instructions"]:
                si = ins.get("sync_info")
                waits = si.get("on_wait") if si else None
                if waits and len(waits) > _MAX_WAITS:
                    for w in waits[:-_MAX_WAITS]:
                        new.append(
                            {
                                "debug": ins.get("debug", 0),
                                "engine": ins["engine"],
                                "ins": [],
                                "name": f"antsw_{ctr}",
                                "opcode": "NoOp",
                                "outs": [],
                                "sync_info": {"on_update": [], "on_wait": [w]},
                            }
                        )
                        ctr += 1
                    si["on_wait"] = waits[-_MAX_WAITS:]
                new.append(ins)
            bb["instructions"] = new
    return bir


def _install_bir_rewrite(nc):
    import orjson

    orig = nc.to_json_bytes

    def patched():
        bir = orjson.loads(orig())
        _split_waits_bir(bir)
        return orjson.dumps(bir)

    nc.to_json_bytes = patched


def build_kernel(t_steps=T, ch_t=8):
    """One-core program; SPMD-replicated across the 8 cores."""
    nc = bass.Bass()
    R = t_steps * BL
    RC = ch_t * BL  # rows per precompute chunk
    NCH = t_steps // ch_t
    NG = NCH  # scan prefetch groups == chunks (both ch_t timesteps)
    scale = 1.0 / math.sqrt(H)

    xt = nc.dram_tensor("xt", [DIN, R], F32, kind="ExternalInput")
    at = nc.dram_tensor("at", [DF, R], F32, kind="ExternalInput")
    wiht = nc.dram_tensor("wiht", [DIN, 3 * H], BF16, kind="ExternalInput")
    wfht = nc.dram_tensor("wfht", [DF, 2 * H], BF16, kind="ExternalInput")
    wkvt = nc.dram_tensor("wkvt", [H, 2 * H], BF16, kind="ExternalInput")
    wcatt = nc.dram_tensor("wcatt", [H, 4 * H], BF16, kind="ExternalInput")
    bs2 = nc.dram_tensor("bs2", [P, 28], F32, kind="ExternalInput")
    bcatn = nc.dram_tensor("bcatn", [P, 128], F32, kind="ExternalInput")
    bqbf = nc.dram_tensor("bqbf", [P, 4], BF16, kind="ExternalInput")
    outd = nc.dram_tensor("outd", [t_steps, P, 4, BL], F32, kind="ExternalOutput")

    from contextlib import ExitStack

    with PatchedTileContext(nc) as tc, ExitStack() as stack:
        consts = stack.enter_context(tc.tile_pool(name="consts", bufs=1))
        wih_sb = consts.tile([P, 16, 3 * H], BF16)
        nc.sync.dma_start(out=wih_sb[:], in_=wiht.rearrange("(k p) m -> p k m", p=P))
        wfh_sb = consts.tile([P, 16, 2 * H], BF16)
        nc.sync.dma_start(out=wfh_sb[:], in_=wfht.rearrange("(k p) m -> p k m", p=P))
        wkv_sb = consts.tile([P, 4, 2 * H], BF16)
        nc.sync.dma_start(out=wkv_sb[:], in_=wkvt.rearrange("(k p) m -> p k m", p=P))
        wcat_sb = consts.tile([P, 4, 4 * H], BF16)
        nc.sync.dma_start(out=wcat_sb[:], in_=wcatt.rearrange("(k p) m -> p k m", p=P))
        bs2_sb = consts.tile([P, 28], F32)
        nc.sync.dma_start(out=bs2_sb[:], in_=bs2[:])
        bcatn_sb = consts.tile([P, 128], F32)
        nc.sync.dma_start(out=bcatn_sb[:], in_=bcatn[:])
        bq_sb = consts.tile([P, 4], BF16)
        nc.sync.dma_start(out=bq_sb[:], in_=bqbf[:])
        ones_dot = consts.tile([P, 1], BF16)
        nc.vector.memset(ones_dot[:], 1.0)
        ones_one = consts.tile([1, 1], BF16)
        nc.vector.memset(ones_one[:], 1.0)
        ones_bc = consts.tile([1, P], BF16)
        nc.vector.memset(ones_bc[:], 1.0)
        # bq . dk per (r/i, t, b), accumulated during precompute
        bqdk_sb = consts.tile([1, 2 * R], BF16)

        dram = stack.enter_context(tc.tile_pool(name="dram", bufs=1, space="DRAM"))
        # [tensor, group, partition, (t, j, b)] — contiguous per partition
        kv_d = dram.tile([6, NG, P, ch_t * 4 * BL], BF16)
        in_d = dram.tile([NG, P, ch_t * 4 * BL], F32)

        # ------------------------- precompute -------------------------
        with (
            tc.tile_pool(name="xa", bufs=2) as xa_pool,
            tc.tile_pool(name="gigf", bufs=2) as gpool,
            tc.tile_pool(name="dpool", bufs=1) as dpool,
            tc.tile_pool(name="stg", bufs=2) as stg_pool,
            tc.tile_pool(name="ppre", bufs=4, space="PSUM") as ppre,
            tc.tile_pool(name="pbq", bufs=2, space="PSUM") as pbq,
        ):
            for c in range(NCH):
                r0 = c * RC
                x_sb = xa_pool.tile([P, 16, RC], BF16, tag="xa")
                nc.gpsimd.dma_start(
                    out=x_sb[:],
                    in_=xt[:, r0 : r0 + RC].rearrange("(k p) r -> p k r", p=P),
                )
                a_sb = xa_pool.tile([P, 16, RC], BF16, tag="xa")
                nc.gpsimd.dma_start(
                    out=a_sb[:],
                    in_=at[:, r0 : r0 + RC].rearrange("(k p) r -> p k r", p=P),
                )
                gi_sb = gpool.tile([P, 8, RC], BF16, tag="gi")
                gf_sb = gpool.tile([P, 8, RC], BF16, tag="gf")
                in_stg = stg_pool.tile([P, ch_t, 4, BL], F32, tag="instg")
                # gi = W_ih @ X^T : m-tiles 0..7 kept (i_r, i_i); 8..11 = i_n
                for m in range(12):
                    ps = ppre.tile([P, RC], F32, tag="ps")
                    for k in range(16):
                        nc.tensor.matmul(
                            ps[:],
                            wih_sb[:, k, m * P : (m + 1) * P],
                            x_sb[:, k, :],
                            start=(k == 0),
                            stop=(k == 15),
                        )
                    if m < 8:
                        nc.vector.tensor_copy(out=gi_sb[:, m, :], in_=ps[:])
                    else:
                        # i_n with bias b_n, straight to scan layout (f32)
                        nc.scalar.activation(
                            out=in_stg[:, :, m - 8, :],
                            in_=ps[:].rearrange("p (t b) -> p t b", b=BL),
                            func=AF.Identity,
                            bias=bs2_sb[:, 24 + (m - 8) : 25 + (m - 8)],
                            scale=1.0,
                        )
                nc.sync.dma_start(
                    out=in_d[c],
                    in_=in_stg[:].rearrange("p t j b -> p (t j b)"),
                )
                # gf = W_fh0 @ A^T : t1 (m 0..3), t2 (m 4..7)
                for m in range(8):
                    ps = ppre.tile([P, RC], F32, tag="ps")
                    for k in range(16):
                        nc.tensor.matmul(
                            ps[:],
                            wfh_sb[:, k, m * P : (m + 1) * P],
                            a_sb[:, k, :],
                            start=(k == 0),
                            stop=(k == 15),
                        )
                    nc.vector.tensor_copy(out=gf_sb[:, m, :], in_=ps[:])
                # d = t - i (both attends), per contraction k-tile
                d_sb = dpool.tile([P, 8, RC], BF16, tag="d")
                for k in range(8):
                    nc.vector.tensor_sub(d_sb[:, k, :], gf_sb[:, k, :], gi_sb[:, k, :])
                # stage2 projections -> scan tensors
                # tau: 0 dk_r, 1 dv_r, 2 v2_r, 3 dk_i, 4 dv_i, 5 v2_i
                srcs = [
                    d_sb[:, 0:4, :],
                    d_sb[:, 0:4, :],
                    gi_sb[:, 0:4, :],
                    d_sb[:, 4:8, :],
                    d_sb[:, 4:8, :],
                    gi_sb[:, 4:8, :],
                ]
                woffs = [0, H, H, 0, H, H]  # Wk for dk, Wv for dv/v2
                for tau in range(6):
                    src = srcs[tau]
                    woff = woffs[tau]
                    stg4 = stg_pool.tile([P, ch_t, 4, BL], BF16, tag=f"s4_{tau}")
                    for m in range(4):
                        ps = ppre.tile([P, RC], F32, tag="ps")
                        for k in range(4):
                            nc.tensor.matmul(
                                ps[:],
                                wkv_sb[:, k, woff + m * P : woff + (m + 1) * P],
                                src[:, k, :],
                                start=(k == 0),
                                stop=(k == 3),
                            )
                        nc.scalar.activation(
                            out=stg4[:, :, m, :],
                            in_=ps[:].rearrange("p (t b) -> p t b", b=BL),
                            func=AF.Identity,
                            bias=bs2_sb[:, tau * 4 + m : tau * 4 + m + 1],
                            scale=1.0,
                        )
                    nc.sync.dma_start(
                        out=kv_d[tau, c],
                        in_=stg4[:].rearrange("p t j b -> p (t j b)"),
                    )
                    if tau in (0, 3):
                        # bqdk[(t,b)] = sum_f bq[f] * dk[f,(t,b)]
                        ri = 0 if tau == 0 else 1
                        psb = pbq.tile([1, RC], F32, tag="psb")
                        for m in range(4):
                            nc.tensor.matmul(
                                psb[:].rearrange("p (t b) -> p t b", b=BL),
                                bq_sb[:, m : m + 1],
                                stg4[:, :, m, :],
                                start=(m == 0),
                                stop=(m == 3),
                            )
                        nc.vector.tensor_copy(
                            out=bqdk_sb[0:1, ri * R + r0 : ri * R + r0 + RC],
                            in_=psb[:],
                        )

        # --------------------------- scan ---------------------------
        with (
            tc.tile_pool(name="scn", bufs=2) as scn,
            tc.tile_pool(name="hpool", bufs=2) as hpool,
            tc.tile_pool(name="work", bufs=2) as work,
            tc.tile_pool(name="pq", bufs=1, space="PSUM") as pq,
            tc.tile_pool(name="phr", bufs=1, space="PSUM") as phr,
            tc.tile_pool(name="phi", bufs=1, space="PSUM") as phi,
            tc.tile_pool(name="phn", bufs=1, space="PSUM") as phn,
            tc.tile_pool(name="psd", bufs=1, space="PSUM") as psd,
            tc.tile_pool(name="ppb", bufs=1, space="PSUM") as ppb,
        ):
            def load_group(g):
                kv = []
                for tau in range(6):
                    kt = scn.tile([P, ch_t, 4, BL], BF16, tag=f"kv{tau}")
                    nc.sync.dma_start(
                        out=kt[:].rearrange("p t j b -> p (t j b)"), in_=kv_d[tau, g]
                    )
                    kv.append(kt)
                it = scn.tile([P, ch_t, 4, BL], F32, tag="inb")
                nc.sync.dma_start(
                    out=it[:].rearrange("p t j b -> p (t j b)"), in_=in_d[g]
                )
                return kv, it

            hx32 = hpool.tile([P, 128], F32, tag="hx32")
            nc.vector.memset(hx32[:], 0.0)
            hxbf = hpool.tile([P, 128], BF16, tag="hxbf")
            nc.vector.memset(hxbf[:], 0.0)

            cur = load_group(0)
            nxt = load_group(1) if NG > 1 else None

            for t in range(t_steps):
                g, tl = divmod(t, ch_t)
                if tl == 0 and g > 0:
                    cur = nxt
                    nxt = load_group(g + 1) if g + 1 < NG else None
                kv, int_ = cur
                dk_r, dv_r, v2_r, dk_i, dv_i, v2_i = (
                    kv[0][:, tl, :, :].rearrange("p j b -> p (j b)"),
                    kv[1][:, tl, :, :].rearrange("p j b -> p (j b)"),
                    kv[2][:, tl, :, :].rearrange("p j b -> p (j b)"),
                    kv[3][:, tl, :, :].rearrange("p j b -> p (j b)"),
                    kv[4][:, tl, :, :].rearrange("p j b -> p (j b)"),
                    kv[5][:, tl, :, :].rearrange("p j b -> p (j b)"),
                )
                i_n = int_[:, tl, :, :].rearrange("p j b -> p (j b)")

                # ---- G = W_cat @ hx (+ per-gate psum tiles) ----
                ps_q = pq.tile([P, 128], F32, tag="psq")
                ps_hr = phr.tile([P, 128], F32, tag="pshr")
                ps_hi = phi.tile([P, 128], F32, tag="pshi")
                ps_hn = phn.tile([P, 128], F32, tag="pshn")

                def gmm(ps, mbase):
                    for j in range(4):
                        for k in range(4):
                            nc.tensor.matmul(
                                ps[:, j * BL : (j + 1) * BL],
                                wcat_sb[
                                    :, k, (mbase + j) * P : (mbase + j + 1) * P
                                ],
                                hxbf[:, k * BL : (k + 1) * BL],
                                start=(k == 0),
                                stop=(k == 3),
                            )

                gmm(ps_q, 12)  # q
                gmm(ps_hr, 0)  # h_r
                gmm(ps_hi, 4)  # h_i

                # ---- attention scores: s = q . dk (+ bq . dk), sigmoid ----
                prod_r = work.tile([P, 128], BF16, tag="prodr")
                nc.vector.tensor_mul(prod_r[:], ps_q[:], dk_r)
                prod_i = work.tile([P, 128], BF16, tag="prodi")
                nc.vector.tensor_mul(prod_i[:], ps_q[:], dk_i)
                ps_s = psd.tile([1, 2 * BL], F32, tag="pss")
                for j in range(4):
                    nc.tensor.matmul(
                        ps_s[0:1, 0:BL],
                        ones_dot[:],
                        prod_r[:, j * BL : (j + 1) * BL],
                        start=(j == 0),
                        stop=False,
                    )
                nc.tensor.matmul(
                    ps_s[0:1, 0:BL],
                    ones_one[:],
                    bqdk_sb[0:1, t * BL : (t + 1) * BL],
                    start=False,
                    stop=True,
                )
                for j in range(4):
                    nc.tensor.matmul(
                        ps_s[0:1, BL : 2 * BL],
                        ones_dot[:],
                        prod_i[:, j * BL : (j + 1) * BL],
                        start=(j == 0),
                        stop=False,
                    )
                nc.tensor.matmul(
                    ps_s[0:1, BL : 2 * BL],
                    ones_one[:],
                    bqdk_sb[0:1, R + t * BL : R + (t + 1) * BL],
                    start=False,
                    stop=True,
                )
                p_sb = work.tile([1, 2 * BL], BF16, tag="psig")
                nc.scalar.activation(
                    out=p_sb[:], in_=ps_s[:], func=AF.Sigmoid, scale=scale
                )
                # broadcast p over partitions via outer product with ones
                ps_p = ppb.tile([P, 2 * 128], F32, tag="psp")
                p_ap = p_sb[:]
                p_rep = bass.AP(
                    tensor=p_ap.tensor,
                    offset=p_ap.offset,
                    ap=[list(p_ap.ap[0]), [BL, 2], [0, 4], [1, BL]],
                )
                nc.tensor.matmul(ps_p[:], ones_bc[:], p_rep, start=True, stop=True)

                gmm(ps_hn, 8)  # h_n

                # ---- gates ----
                base_r = work.tile([P, 128], BF16, tag="baser")
                nc.vector.tensor_add(base_r[:], ps_hr[:], v2_r)
                base_i = work.tile([P, 128], BF16, tag="basei")
                nc.vector.tensor_add(base_i[:], ps_hi[:], v2_i)
                r1 = work.tile([P, 128], BF16, tag="r1")
                nc.vector.tensor_mul(r1[:], ps_p[:, 0:128], dv_r)
                i1 = work.tile([P, 128], BF16, tag="i1")
                nc.vector.tensor_mul(i1[:], ps_p[:, 128:256], dv_i)
                rpre = work.tile([P, 128], BF16, tag="rpre")
                nc.vector.tensor_add(rpre[:], r1[:], base_r[:])
                ipre = work.tile([P, 128], BF16, tag="ipre")
                nc.vector.tensor_add(ipre[:], i1[:], base_i[:])
                rg = work.tile([P, 128], BF16, tag="rg")
                nc.scalar.activation(out=rg[:], in_=rpre[:], func=AF.Sigmoid, scale=1.0)
                ig = work.tile([P, 128], BF16, tag="ig")
                nc.scalar.activation(out=ig[:], in_=ipre[:], func=AF.Sigmoid, scale=1.0)
                igc = work.tile([P, 128], BF16, tag="igc")
                nc.scalar.activation(
                    out=igc[:], in_=ipre[:], func=AF.Sigmoid, scale=-1.0
                )
                g_n = work.tile([P, 128], BF16, tag="gn")
                nc.vector.tensor_add(g_n[:], ps_hn[:], bcatn_sb[:])
                np1 = work.tile([P, 128], BF16, tag="np1")
                nc.vector.tensor_mul(np1[:], rg[:], g_n[:])
                npre = work.tile([P, 128], F32, tag="npre")
                nc.vector.tensor_add(npre[:], np1[:], i_n)
                ng = work.tile([P, 128], F32, tag="ng")
                nc.scalar.activation(out=ng[:], in_=npre[:], func=AF.Tanh, scale=1.0)
                t1 = work.tile([P, 128], F32, tag="t1")
                nc.vector.tensor_mul(t1[:], ig[:], hx32[:])
                t3 = work.tile([P, 128], F32, tag="t3")
                nc.vector.tensor_mul(t3[:], igc[:], ng[:])
                hy32 = hpool.tile([P, 128], F32, tag="hx32")
                nc.vector.tensor_add(hy32[:], t1[:], t3[:])
                hybf = hpool.tile([P, 128], BF16, tag="hxbf")
                nc.vector.tensor_copy(out=hybf[:], in_=hy32[:])

                nc.sync.dma_start(
                    out=outd[t].rearrange("p j b -> p (j b)"),
                    in_=hy32[:],
                )
                hx32, hxbf = hy32, hybf
    _install_bir_rewrite(nc)
    return nc


# ------------------------ host-side marshaling ------------------------

def _prep(inputs, t_steps=T):
    inp = inputs
    W_ih, b_ih = np.asarray(inp["W_ih"]), np.asarray(inp["b_ih"])
    W_fh0, b_fh0 = np.asarray(inp["W_fh0"]), np.asarray(inp["b_fh0"])
    W_hh, b_hh = np.asarray(inp["W_hh"]), np.asarray(inp["b_hh"])
    Wq, bq = np.asarray(inp["Wq"]), np.asarray(inp["bq"])
    Wk = np.asarray(inp["Wk"])
    Wv, bv = np.asarray(inp["Wv"]), np.asarray(inp["bv"])

    b_r, b_i, b_n = b_ih[:H], b_ih[H : 2 * H], b_ih[2 * H :]
    c1, c2 = b_fh0[:H], b_fh0[H:]
    bhr, bhi, bhn = b_hh[:H], b_hh[H : 2 * H], b_hh[2 * H :]

    wiht = np.ascontiguousarray(W_ih.T).astype(BF)
    wfht = np.ascontiguousarray(W_fh0.T).astype(BF)
    wkvt = np.concatenate([Wk.T, Wv.T], axis=1).astype(BF)
    wcatt = np.concatenate([W_hh.T, Wq.T], axis=1).astype(BF)

    def col(v):  # [512] -> [128, 4] per-partition m-tile columns
        return v.reshape(4, P).T

    bias_cols = np.concatenate(
        [
            col(Wk @ (c1 - b_r)),
            col(Wv @ (c1 - b_r)),
            col(Wv @ b_r + bv + bhr),
            col(Wk @ (c2 - b_i)),
            col(Wv @ (c2 - b_i)),
            col(Wv @ b_i + bv + bhi),
            col(b_n),
        ],
        axis=1,
    ).astype(np.float32)  # [128, 28]
    bcatn = np.repeat(bhn.reshape(4, P).T[:, :, None], BL, axis=2).reshape(
        P, 128
    ).astype(np.float32)
    bqbf = bq.reshape(4, P).T.astype(BF)

    shared = dict(
        wiht=wiht, wfht=wfht, wkvt=wkvt, wcatt=wcatt,
        bs2=bias_cols, bcatn=bcatn, bqbf=bqbf,
    )
    x = np.asarray(inp["input_feats"])[:, :t_steps]
    a = np.asarray(inp["aux0"])[:, :t_steps]
    in_maps = []
    for c in range(NCORES):
        xc = x[c * BL : (c + 1) * BL]  # [BL, T, DIN]
        ac = a[c * BL : (c + 1) * BL]
        xtc = np.ascontiguousarray(
            xc.transpose(2, 1, 0).reshape(DIN, t_steps * BL)
        ).astype(np.float32)
        atc = np.ascontiguousarray(
            ac.transpose(2, 1, 0).reshape(DF, t_steps * BL)
        ).astype(np.float32)
        in_maps.append(dict(xt=xtc, at=atc, **shared))
    return in_maps


def _assemble(results, t_steps=T):
    outs = []
    for c in range(NCORES):
        od = results[c]["outd"]  # [T, 128, 4, BL]
        outs.append(od.transpose(3, 0, 2, 1).reshape(BL, t_steps, H))
    out = np.concatenate(outs, axis=0).astype(np.float32)  # [B, T, H]
    return out, np.ascontiguousarray(out[:, -1, :])


_RUNNER = None


def _get_runner():
    """Build the Bass program once and wrap it in a persistent jitted
    shard_map callable (same lowering run_bass_kernel_spmd uses under axon,
    but reusable across calls without re-tracing/compiling)."""
    global _RUNNER
    if _RUNNER is not None:
        return _RUNNER
    import jax
    from jax.sharding import Mesh, PartitionSpec
    from concourse import bass2jax, mybir as mb

    try:
        from jax.experimental.shard_map import shard_map
    except ImportError:
        from jax.shard_map import shard_map

    nc = build_kernel()
    bass2jax.install_neuronx_cc_hook()

    partition_name = (
        nc.partition_id_tensor.name if nc.partition_id_tensor is not None else None
    )
    in_names, out_names, out_avals = [], [], []
    for alloc in nc.m.functions[0].allocations:
        if not isinstance(alloc, mb.MemoryLocationSet):
            continue
        name = alloc.memorylocations[0].name
        if alloc.kind == "ExternalInput":
            if name != partition_name:
                in_names.append(name)
        elif alloc.kind == "ExternalOutput":
            out_names.append(name)
            out_avals.append(
                jax.core.ShapedArray(
                    tuple(alloc.tensor_shape), mb.dt.np(alloc.dtype)
                )
            )
    n_params = len(in_names)
    all_names = list(in_names) + list(out_names)
    if partition_name is not None:
        all_names.append(partition_name)

    def _body(*args):
        operands = list(args)
        if partition_name is not None:
            operands.append(bass2jax.partition_id_tensor())
        outs = bass2jax._bass_exec_p.bind(
            *operands,
            out_avals=tuple(out_avals),
            in_names=tuple(all_names),
            out_names=tuple(out_names),
            lowering_input_output_aliases=(),
            sim_require_finite=True,
            sim_require_nnan=True,
            nc=nc,
        )
        return tuple(outs)

    devices = jax.devices()[:NCORES]
    mesh = Mesh(np.asarray(devices), ("core",))
    n_outs = len(out_names)
    sharded = jax.jit(
        shard_map(
            _body,
            mesh=mesh,
            in_specs=(PartitionSpec("core"),) * (n_params + n_outs),
            out_specs=(PartitionSpec("core"),) * n_outs,
            check_rep=False,
        ),
        donate_argnums=tuple(range(n_params, n_params + n_outs)),
        keep_unused=True,
    )
    _RUNNER = (sharded, in_names, out_names, out_avals)
    return _RUNNER


def _run_device(in_maps):
    sharded, in_names, out_names, out_avals = _get_runner()
    concat_in = [
        np.concatenate([np.asarray(in_maps[c][nm]) for c in range(NCORES)], axis=0)
        for nm in in_names
    ]
    concat_zeros = [
        np.zeros((NCORES * a.shape[0], *a.shape[1:]), a.dtype) for a in out_avals
    ]
    out_arrs = sharded(*concat_in, *concat_zeros)
    results = [
        {
            nm: np.asarray(out_arrs[i]).reshape(NCORES, *out_avals[i].shape)[c]
            for i, nm in enumerate(out_names)
        }
        for c in range(NCORES)
    ]
    return results


def kernel(**inputs):
    in_maps = _prep(inputs)
    results = _run_device(in_maps)
    return _assemble(results)
